# revision 46
# baseline (speedup 1.0000x reference)
"""Mie scattering phase function on 8 Trainium2 NeuronCores — v2.

Math: the reference's S1/S2 amplitudes are polynomials in mu of degree <= NMAX+1.
Parity-split S(mu) = E(mu^2) + mu*O(mu^2); fit the 8 scalar targets
(E/O parts of Re/Im S1/S2, pre-scaled by 1/sqrt(2 x^2)) in a two-level Chebyshev
product basis on uh = 2 mu^2 - 1:  { T_j(T_8(uh)) * T_r(uh) : j,r in 0..7 }
(64 features, spans degree <= 63 in uh = 126 in mu; NMAX=135 harmonics decay
fast enough past ~127 that the residual is ~1e-4).

Device kernel per core (131072 angles = 2 megatiles of 128x512 angle columns):
  - Feature planes F[p, k*512 + c] fp16 (k = j*8+r "plane-major"): the 7
    j-level products run on DVE in 2x fp16 mode (outer-dim broadcast of Tv_j
    keeps the last AP pair packed) — 2.2us per [128,4096] op vs 8.2us on
    GPSIMD in the old k-inner layout.
  - Chebyshev base chain in f32 planes (binary doubling), squares on ACT,
    muls/affines/stt on GPSIMD, converted to fp16 in two batched ACT copies.
  - Transposes: fp32-bitcast PAIR-PACKED PE transposes — each [128,128] f32
    transpose moves 4 angle-columns x 64 bands (bit-exact on HW, probed):
    128 transposes per megatile instead of 256 fp16 ones. 4 per PSUM bank,
    single [128,512]f32 copyback per bank (DVE/ACT alternating).
  - Matmuls: per 4-col unit, two parity-split matmuls lhsT = ftT[qb, par]
    (a-stride-2 fp16 view, M=128 angle rows), rhs = packed C [128,16]
    (block-diagonal: rows 0-63 -> outputs 0-7 for even col-pair, 64-127 ->
    8-15 for odd). PSUM f32; 64 slots (2 banks) per drain group.
  - Epilogue per drain group: S_t = E_t + mu*O_t batched, square on ACT,
    pairwise adds -> phase; cols mapped 128g + 4i + 2u + par.
"""
import math
from contextlib import ExitStack

import numpy as np

NMAX = 135
R = 8
J = 8
NFEAT = R * J  # 64
N_ANGLES = 1048576
N_CORES = 8
PER_CORE = N_ANGLES // N_CORES  # 131072
P = 128
MEGA_COLS = 512                   # angle columns per megatile
N_MEGA = PER_CORE // (P * MEGA_COLS)  # 2
NUNIT = MEGA_COLS // 4            # 4-col units per megatile (128)
UNITS_PER_BANK = 4                # transposes per PSUM bank
N_BANKS = NUNIT // UNITS_PER_BANK  # 32
UNITS_PER_DRAIN = 32              # units per PSUM drain group (2 banks, 128 cols)
N_DRAIN = NUNIT // UNITS_PER_DRAIN  # 4
# feature-store column chunks per megatile, in c2 (column-pair) units.
# chunk 0 is small so the first transposes are gated on only a sliver of
# the conversion+product work (shortens the exposed lead-in).
CH_RANGES_MT0 = ((0, 128), (128, 256))
CH_RANGES_MT1 = ((0, 128), (128, 256))


# ----------------------------------------------------------------------------
# Host-side math (float64): replicate the reference recurrences + basis fit
# ----------------------------------------------------------------------------

def _riccati_f64(z):
    z = complex(z)
    inv = 1.0 / (z + 1e-12)
    psi = np.zeros(NMAX + 2, np.complex128)
    chi = np.zeros(NMAX + 2, np.complex128)
    psi[0] = np.sin(z)
    psi[1] = psi[0] * inv - np.cos(z)
    chi[0] = -np.cos(z)
    chi[1] = np.cos(z) * inv - np.sin(z)
    for n in range(2, NMAX + 2):
        coef = (2.0 * n - 1.0) * inv
        psi[n] = coef * psi[n - 1] - psi[n - 2]
        chi[n] = coef * chi[n - 1] - chi[n - 2]
    xi = psi - 1j * chi
    i = np.arange(1, NMAX + 1, dtype=np.float64)
    psi_prime = np.concatenate([[np.cos(z)], psi[:NMAX] - i * inv * psi[1:NMAX + 1]])
    xi_prime = np.concatenate(
        [[np.cos(z) + 1j * np.sin(z)], xi[:NMAX] - i * inv * xi[1:NMAX + 1]])
    return psi, xi, psi_prime, xi_prime


def _mie_ab_f64(x, m):
    psi, xi, psip, xip = _riccati_f64(x)
    psm, _, psmp, _ = _riccati_f64(m * x)
    s = slice(1, NMAX + 1)
    an = (m * psm[s] * psip[s] - psi[s] * psmp[s]) / \
         (m * psm[s] * xip[s] - xi[s] * psmp[s] + 1e-30)
    bn = (psm[s] * psip[s] - m * psi[s] * psmp[s]) / \
         (psm[s] * xip[s] - m * xi[s] * psmp[s] + 1e-30)
    return an, bn


def _s1s2_f64(mu, x, m):
    an, bn = _mie_ab_f64(x, m)
    n = np.arange(1, NMAX + 1, dtype=np.float64)
    f = (2.0 * n + 1.0) / (n * (n + 1.0))
    fa, fb = f * an, f * bn
    mu = np.asarray(mu, np.float64)
    pi_all = np.zeros((NMAX, mu.size))
    tau_all = np.zeros((NMAX, mu.size))
    pi_all[0] = 3.0 * mu
    tau_all[0] = mu
    p1, p2 = 3.0 * mu, np.ones_like(mu)
    for k in range(2, NMAX + 1):
        nn = float(k)
        p = ((2 * nn + 1) * mu * p1 - (nn + 1) * p2) / nn
        t = nn * mu * p - (nn + 1) * p1
        pi_all[k - 1] = p
        tau_all[k - 1] = t
        p1, p2 = p, p1
    S1 = fa @ pi_all + fb @ tau_all
    S2 = fa @ tau_all + fb @ pi_all
    return S1, S2


def _cheb_T(k, x):
    return np.cos(k * np.arccos(np.clip(x, -1.0, 1.0)))


def _fit_coeffs(wavelength, radius, m_real, m_imag):
    """Returns C (NFEAT, 8) float64 — columns: E/O of S1r,S1i,S2r,S2i scaled."""
    x = 2.0 * math.pi * radius / (wavelength * 1e-9)
    m = m_real + 1j * m_imag
    M = 1024
    uh = np.cos((np.arange(M) + 0.5) * np.pi / M)
    u = (uh + 1.0) / 2.0
    mu = np.sqrt(u)
    S1p, S2p = _s1s2_f64(mu, x, m)
    S1m, S2m = _s1s2_f64(-mu, x, m)
    scale = 1.0 / math.sqrt(2.0 * x * x)
    targets = []
    pairs = ((S1p.real, S1m.real), (S1p.imag, S1m.imag),
             (S2p.real, S2m.real), (S2p.imag, S2m.imag))
    for Sp, Sm in pairs:                       # E parts: columns 0..3
        targets.append((Sp + Sm) / 2.0 * scale)
    for Sp, Sm in pairs:                       # O parts: columns 4..7
        targets.append((Sp - Sm) / (2.0 * mu + 1e-300) * scale)
    T = np.stack([_cheb_T(r, uh) for r in range(R + 1)])
    Tv = np.stack([_cheb_T(j, T[R]) for j in range(J)])
    B = np.zeros((M, NFEAT))
    for j in range(J):
        for r in range(R):
            B[:, j * R + r] = Tv[j] * T[r]
    C, *_ = np.linalg.lstsq(B, np.stack(targets, axis=1), rcond=None)
    return C


# ----------------------------------------------------------------------------
# Device kernel (Bass / Tile)
# ----------------------------------------------------------------------------

_CACHE = {}


def _build_nc(reps=1, feed_n=4, cb_pat="ADADADADADADADAD", red_keep=1):
    """cb_pat: per-copyback engine pattern, cycled (D=DVE, A=ACT)."""
    import concourse.bass as bass
    import concourse.mybir as mybir
    import concourse.tile as tile
    from concourse import bacc, masks

    f32 = mybir.dt.float32
    f16 = mybir.dt.float16
    AOP = mybir.AluOpType
    ACTF = mybir.ActivationFunctionType

    nc = bacc.Bacc("TRN2", target_bir_lowering=False, debug=False)
    mu_d = nc.dram_tensor("mu", [PER_CORE], f32, kind="ExternalInput").ap()
    cpk_d = nc.dram_tensor("cpk", [P, 16], f16, kind="ExternalInput").ap()
    ph_d = nc.dram_tensor("phase", [PER_CORE], f32, kind="ExternalOutput").ap()

    mu_v = mu_d.rearrange("(m p c) -> m p c", p=P, c=MEGA_COLS)
    ph_v = ph_d.rearrange("(m p c) -> m p c", p=P, c=MEGA_COLS)

    with tile.TileContext(nc) as tc, ExitStack() as ctx:
        const_p = ctx.enter_context(tc.tile_pool(name="const", bufs=1))
        fstore_p = ctx.enter_context(tc.tile_pool(name="fstore", bufs=2))
        ch_p = ctx.enter_context(tc.tile_pool(name="chain", bufs=1))
        mu_p = ctx.enter_context(tc.tile_pool(name="mu", bufs=2))
        ph_p = ctx.enter_context(tc.tile_pool(name="ph", bufs=2))
        tmp_p = ctx.enter_context(tc.tile_pool(name="tmp", bufs=1))
        sq_p = ctx.enter_context(tc.tile_pool(name="sq", bufs=2))
        ftT_p = ctx.enter_context(tc.tile_pool(name="ftT", bufs=3))
        psum_mm = ctx.enter_context(tc.tile_pool(name="psmm", bufs=2, space="PSUM"))
        psum_tr = ctx.enter_context(tc.tile_pool(name="pstr", bufs=2, space="PSUM"))

        cpk_sb = const_p.tile([P, 16], f16)
        two_sb = const_p.tile([P, 1], f32)
        ident32 = const_p.tile([P, P], f32)

        def emit_consts():
            nc.sync.dma_start(cpk_sb[:], cpk_d)
            nc.gpsimd.memset(two_sb[:], 2.0)
            masks.make_identity(nc, ident32[:])

        # guards: last epilogue instruction that read each psum_mm buf
        guard = [None, None]

        rep_cm = tc.For_i(0, reps, 1) if reps > 1 else None
        if rep_cm is not None:
            rep_cm.__enter__()
            tc.strict_bb_all_engine_barrier()

        def make_stage_a(mt):
            """Thunks for megatile mt's DMA + f32 Chebyshev chain + fp16
            conversions + DVE 2x plane products. Features are stored in
            NCH column-chunks so the first transposes can start after only
            1/NCH of the conversion+product work."""
            ops = []
            mu_t = mu_p.tile([P, MEGA_COLS], f32)
            ch = ch_p.tile([P, 14 * MEGA_COLS], f32, tag="ch")
            ch3 = ch[:].rearrange("p (n c) -> p n c", n=14)
            tv16 = ch_p.tile([P, 7 * MEGA_COLS], f16, tag="tv16")
            tv3 = tv16[:].rearrange("p (n c) -> p n c", n=7)
            FBIG = fstore_p.tile([P, NFEAT * MEGA_COLS], f16)
            # pair-interleaved layout: [p, c2, k, par]
            # (par = column parity; (par0,par1) fp16 pairs form one f32 so a
            # PE transpose with a single contiguous free run moves 4 cols)
            FBh = FBIG[:].rearrange("p (c2 k par) -> p c2 k par",
                                    k=NFEAT, par=2)
            ch4 = ch[:].rearrange("p (n c2 par) -> p c2 n par", n=14, par=2)
            tv4 = tv16[:].rearrange("p (n c2 par) -> p n c2 par", n=7, par=2)
            st = {"mu": mu_t, "F": FBIG}

            ops.append(lambda: nc.sync.dma_start(
                mu_t[:, 0:MEGA_COLS // 2], mu_v[mt, :, 0:MEGA_COLS // 2]))
            ops.append(lambda: nc.scalar.dma_start(
                mu_t[:, MEGA_COLS // 2:], mu_v[mt, :, MEGA_COLS // 2:]))

            # plane indices in ch: 0..6 = T1..T7; 7..13 = T8,T16,T24,T32,T40,T48,T56
            PL = {1: 0, 2: 1, 3: 2, 4: 3, 5: 4, 6: 5, 7: 6,
                  8: 7, 16: 8, 24: 9, 32: 10, 40: 11, 48: 12, 56: 13}

            def pl(m):
                return ch3[:, PL[m]]

            # Chain in DOUBLED Chebyshev values D_m = 2*T_m(uh):
            #   D_{2m}  = D_m^2 - 2          (mul DVE + scalar-add DVE)
            #   D_{m+n} = D_m*D_n - D_{m-n}  (mul + subtract, both Pool TT)
            # The 0.5 back-scale folds into the fp16 conversions' ACT scale.
            # Critical dbl path stays on DVE (no cross-engine sem latency);
            # side adds live entirely on GPSIMD.
            u_t = tmp_p.tile([P, MEGA_COLS], f32, tag="utile")
            ops.append(lambda: nc.vector.tensor_mul(u_t[:], mu_t[:], mu_t[:]))
            ops.append(lambda: nc.vector.tensor_scalar(
                pl(1), u_t[:], 4.0, -2.0, AOP.mult, AOP.add))

            def dbl(m):
                # D_2m = D_m^2 - 2: mul + scalar-add, both DVE (critical path)
                sq = tmp_p.tile([P, MEGA_COLS], f32, tag=f"sq{(2 * m) % 3}")
                ops.append(lambda: nc.vector.tensor_mul(sq[:], pl(m), pl(m)))
                ops.append(lambda: nc.vector.tensor_scalar_add(
                    pl(2 * m), sq[:], -2.0))

            def add_(m, n, eng=None):
                # D_{m+n} = D_m*D_n - D_{m-n}: mul + sub (GPSIMD off-path;
                # DVE when the result is on the critical dbl tail)
                e = eng or nc.gpsimd
                pr = tmp_p.tile([P, MEGA_COLS], f32, tag=f"pr{(m + n) % 3}")
                ops.append(lambda: e.tensor_mul(pr[:], pl(m), pl(n)))
                ops.append(lambda: e.tensor_tensor(
                    pl(m + n), pr[:], pl(m - n), AOP.subtract))

            def conv2(m, i):
                # Tv plane -> fp16 right after it is produced (ACT 0.5x copy)
                ops.append(lambda: nc.scalar.activation(
                    tv3[:, i], pl(m), ACTF.Copy, bias=0.0, scale=0.5))

            dbl(1)        # T2
            add_(2, 1)    # T3
            dbl(2)        # T4
            add_(3, 2)    # T5
            dbl(3)        # T6
            add_(4, 3)    # T7
            dbl(4)        # T8
            conv2(8, 0)
            dbl(8)        # T16
            conv2(16, 1)
            add_(16, 8, eng=nc.vector)   # T24 (critical: feeds T48/T56)
            conv2(24, 2)
            dbl(16)       # T32
            conv2(32, 3)
            add_(24, 16)  # T40
            conv2(40, 4)
            dbl(24)       # T48
            conv2(48, 5)
            add_(32, 24)  # T56
            conv2(56, 6)

            # per column-chunk: memset ones, conv1, then the 7 products
            for (ca, cb) in (CH_RANGES_MT0 if mt == 0 else CH_RANGES_MT1):
                cw2 = cb - ca
                ops.append(lambda ca=ca, cb=cb:
                           nc.gpsimd.memset(FBh[:, ca:cb, 0, :], 1.0))
                ops.append(lambda ca=ca, cb=cb:
                           nc.scalar.activation(FBh[:, ca:cb, 1:8, :],
                                                ch4[:, ca:cb, 0:7, :],
                                                ACTF.Copy, bias=0.0, scale=0.5))
                for j in range(1, J):
                    tvb = tv4[:, j - 1:j, ca:cb, :].rearrange(
                        "p one c2 par -> p c2 one par").broadcast_to(
                        [P, cw2, 8, 2])
                    ops.append(lambda ca=ca, cb=cb, j=j, tvb=tvb:
                               nc.vector.tensor_mul(
                                   FBh[:, ca:cb, 8 * j:8 * j + 8, :], tvb,
                                   FBh[:, ca:cb, 0:8, :]))
            return ops, st

        def stage_b(mt, st, a_feed):
            """Transpose/matmul/drain pipeline for megatile mt."""
            mu_t, FBIG = st["mu"], st["F"]
            F32 = FBIG[:].bitcast(f32)                     # [P, 16384]
            # [p, unit(128 global), 128 f32] — contiguous per unit
            trin = F32.rearrange("p (q m) -> p q m", m=2 * NFEAT)

            ps = None
            pstr = None
            ftTq = [None] * (N_BANKS // 2)   # copyback tiles per 2-bank group
            cb_n = 0

            def issue_mms(b2):
                """Issue the 16 matmuls for group b2 (units 8*b2..8*b2+7)."""
                nonlocal prev_mm
                ftT = ftTq[b2]
                vv = ftT[:].rearrange("p (qb a par) -> p qb a par", qb=8, par=2)
                for qb in range(8):
                    q = 8 * b2 + qb
                    g, i = q // UNITS_PER_DRAIN, q % UNITS_PER_DRAIN
                    for par in range(2):
                        slot = par * 32 + i
                        start = (i == 0)
                        mm = nc.tensor.matmul(
                            ps4[g % 2][:, slot], vv[:, qb, :, par], cpk_sb[:],
                            start=start, stop=(i == 31))
                        if start and guard[g % 2] is not None:
                            tile.add_dep_helper(mm.ins, guard[g % 2].ins,
                                                sync=True,
                                                reason="bank reuse after epi")
                        if not start and prev_mm is not None:
                            tile.add_dep_helper(mm.ins, prev_mm.ins, sync=False,
                                                reason="psum bank order")
                        prev_mm = mm

            prev_mm = None
            prev_tr = None
            ps4 = [None, None]
            NB2 = N_BANKS // 2   # 2-bank transpose groups (8 units each)
            for b2 in range(NB2):
                g = (8 * b2) // UNITS_PER_DRAIN
                if (8 * b2) % UNITS_PER_DRAIN == 0:
                    pst = psum_mm.tile([P, UNITS_PER_DRAIN * 2 * 16], f32)
                    ps4[g % 2] = pst[:].rearrange("p (s n) -> p s n", n=16)
                    ps = pst
                # 8 packed transposes into one 2-bank psum_tr tile
                pstr = psum_tr.tile([P, 1024], f32)
                for s in range(8):
                    q = 8 * b2 + s
                    tr = nc.tensor.matmul(
                        pstr[:, s * P:(s + 1) * P], trin[:, q],
                        ident32[:], is_transpose=True,
                        start=(s % 4 == 0), stop=(s % 4 == 3))
                    if s % 4 != 0:
                        tile.add_dep_helper(tr.ins, prev_tr.ins, sync=False,
                                            reason="transpose bank order")
                    prev_tr = tr
                # copyback 2 banks -> SBUF per cb_pat
                ftTq[b2] = ftT_p.tile([P, 2048], f16, name="ftT", tag="ftT")
                if cb_pat[cb_n % len(cb_pat)] == "D":
                    nc.vector.tensor_copy(ftTq[b2][:], pstr[:].bitcast(f16))
                else:
                    nc.scalar.copy(ftTq[b2][:], pstr[:].bitcast(f16))
                cb_n += 1

                # matmuls for group b2-2 (lag-2 hides copyback latency)
                if b2 >= 2:
                    issue_mms(b2 - 2)
                for _ in range(feed_n):
                    a_op = next(a_feed, None)
                    if a_op is not None:
                        a_op()

                # drain when the last matmuls of psum group gq were issued
                if b2 >= 2 and (8 * (b2 - 2) + 7) % UNITS_PER_DRAIN == UNITS_PER_DRAIN - 1:
                    gq = (8 * (b2 - 2)) // UNITS_PER_DRAIN
                    drain(mt, gq, mu_t, ps4[gq % 2])

            # tail
            for b2 in (NB2 - 2, NB2 - 1):
                issue_mms(b2)
                if (8 * b2 + 7) % UNITS_PER_DRAIN == UNITS_PER_DRAIN - 1:
                    gq = (8 * b2) // UNITS_PER_DRAIN
                    drain(mt, gq, mu_t, ps4[gq % 2])

        ph_tiles = {}
        pending_red = []

        def flush_red(keep=1):
            while len(pending_red) > keep:
                rmt, rg, phw, sq4 = pending_red.pop(0)
                nc.vector.tensor_reduce(phw, sq4, mybir.AxisListType.X, AOP.add)
                c0, c1 = 128 * rg, 128 * (rg + 1)
                nc.scalar.dma_start(ph_v[rmt, :, c0:c1],
                                    ph_tiles[rmt][:, c0:c1])

        def drain(mt, g, mu_t, psg):
            flush_red(keep=red_keep)
            """Epilogue for drain group g: cols 128g..128g+127.
            psg: [P, 64, 16] f32 view; slot = i*2+par, n = u*8+t."""
            # slot = par*32 + i; n = u*8 + t. View as [p, par, iu(64), t]
            # (iu = i*2+u merges contiguously; col = 4i + 2u + par).
            pp = psg.rearrange("p (par i) (u t) -> p par (i u) t", par=2, u=2)
            pe = pp[:, :, :, 0:4]
            po = pp[:, :, :, 4:8]
            mu4 = mu_t[:, 128 * g:128 * (g + 1)].rearrange(
                "p (iu par) -> p par iu", par=2).unsqueeze(-1) \
                .broadcast_to([P, 2, 64, 4])
            s_t = sq_p.tile([P, 512], f32, tag="stile")
            s4 = s_t[:].rearrange("p (par iu t) -> p par iu t", par=2, t=4)
            sqt = sq_p.tile([P, 512], f32, tag="sqtile")
            if g == 0:
                ph_tiles[mt] = ph_p.tile([P, MEGA_COLS], f32, name="ph", tag="ph")
            ph_t = ph_tiles[mt]
            nc.vector.tensor_mul(s4, mu4, po)
            guard[g % 2] = nc.vector.tensor_add(s4, s4, pe)
            nc.gpsimd.tensor_mul(sqt[:], s_t[:], s_t[:])
            # phase = sum over t — DEFERRED one group so DVE's reduce never
            # stalls on the Pool square round-trip
            sq4 = sqt[:].rearrange("p (c t) -> p c t", t=4)
            phw = ph_t[:, 128 * g:128 * (g + 1)].rearrange(
                "p (iu par) -> p par iu", par=2)
            pending_red.append((mt, g, phw, sq4))

        a_ops, a_st = make_stage_a(0)
        a_ops[0]()   # mu DMA halves first — they gate the whole lead-in
        a_ops[1]()
        emit_consts()
        for a_op in a_ops[2:]:
            a_op()
        for mt in range(N_MEGA):
            if mt + 1 < N_MEGA:
                n_ops, n_st = make_stage_a(mt + 1)
            else:
                n_ops, n_st = [], None
            feed = iter(n_ops)
            stage_b(mt, a_st, feed)
            for a_op in feed:
                a_op()
            a_st = n_st

        flush_red(keep=0)

        if rep_cm is not None:
            rep_cm.__exit__(None, None, None)

    nc.compile()
    return nc


def _get_compiled():
    if "nc" not in _CACHE:
        _CACHE["nc"] = _build_nc()
    return _CACHE["nc"]


def _make_in_maps(mu, wavelength, radius, m_real, m_imag):
    C = _fit_coeffs(wavelength, radius, m_real, m_imag)
    cpk = np.zeros((P, 16), np.float16)
    cpk[0:NFEAT, 0:8] = C.astype(np.float16)
    cpk[NFEAT:2 * NFEAT, 8:16] = C.astype(np.float16)
    shards = mu.reshape(N_CORES, PER_CORE)
    return [{"mu": shards[i], "cpk": cpk} for i in range(N_CORES)]


def kernel(cos_theta, wavelength, radius, m_real, m_imag):
    from concourse.bass_utils import run_bass_kernel_spmd

    mu = np.asarray(cos_theta, np.float32).reshape(-1)
    assert mu.size == N_ANGLES
    in_maps = _make_in_maps(mu, float(np.asarray(wavelength)),
                            float(np.asarray(radius)),
                            float(np.asarray(m_real)),
                            float(np.asarray(m_imag)))
    nc = _get_compiled()
    import os
    trace = bool(os.environ.get("MIE_TRACE"))
    res = run_bass_kernel_spmd(nc, in_maps, list(range(N_CORES)), trace=trace)
    _CACHE["last_res"] = res
    out = np.concatenate([np.asarray(res.results[i]["phase"], np.float32)
                          for i in range(N_CORES)])
    return out
